# revision 1
# baseline (speedup 1.0000x reference)
"""Trainium2 Bass kernel for ContinuousTimeAwareMHSA.

Full inputs in, full outputs out. Sharding: 8 cores = 4 batches x 2 head
groups (8 heads each). Per core the kernel computes, for batch b and
head-group g, out[b, :, g*512:(g+1)*512].

Math per core (S=2048, HID=1024, DG=512, D=64, HL=8 local heads):
  QT = (x @ Wq[:, g])^T   [DG, S]   (fp32r matmuls, layout d-major)
  KT = (x @ Wk[:, g])^T   [DG, S]
  V  =  x @ Wv[:, g]      [S, DG]   (bf16, with a ones column per head)
  G  = mask * exp(-|alpha| * ti)    [S, S] transposed to [k, q], bf16
  per head h, scores^T[k, q] = K_h Q_h^T / 8 : pm = exp(sT/8) * G
  O'[d+1, q] = V'_h^T pm  (row d = sum_k pm = softmax denominator)
  out = O'[0:d] / O'[d]   (transposed back via PE)

softmax skips max-subtraction: scores/8 have unit-ish scale (|s|<~10),
so exp never overflows and softmax is shift-invariant.
"""

import sys

for p in ("/opt/trn_rl_repo",):
    if p not in sys.path:
        sys.path.insert(0, p)

from contextlib import ExitStack

import numpy as np

import concourse.bass as bass
import concourse.tile as tile
from concourse import bacc, mybir
from concourse.masks import make_identity

F32 = mybir.dt.float32
F32R = mybir.dt.float32r
BF16 = mybir.dt.bfloat16
I32 = mybir.dt.int32
EXP = mybir.ActivationFunctionType.Exp
COPY = mybir.ActivationFunctionType.Copy

N_CORES = 8


def build_nc(S, HID, DG, D, alpha, num_devices=N_CORES, qg_size=None):
    """Build the per-core SPMD program. All cores run the same program on
    different shards. alpha is baked in as an immediate."""
    NHC = HID // 128      # hidden contraction chunks
    NSB = S // 128        # s blocks (also k chunks NKC)
    NDG = DG // 128       # d blocks of the local head group
    HL = DG // D          # local heads
    HPD = 128 // D        # heads per d-block
    HL65 = HL * (D + 1)
    QG = qg_size or min(1024, S)  # q-group size for phase B
    NQG = S // QG
    JW = min(512, QG)     # matmul N chunk
    NJ = QG // JW
    G4 = min(4, NHC)      # transpose batch for XbT
    KHW = min(512, S)     # G-build k-chunk width
    NKH = S // KHW
    NKC = NSB
    SGW = min(512, S)     # s-group width for phase A
    NSG = S // SGW
    SB_G = SGW // 128     # s blocks per s group

    nc = bacc.Bacc("TRN2", target_bir_lowering=False, debug=False,
                   num_devices=num_devices)

    x_d = nc.dram_tensor("x", [S, HID], F32, kind="ExternalInput").ap()
    wq_d = nc.dram_tensor("wq", [HID, DG], F32, kind="ExternalInput").ap()
    wk_d = nc.dram_tensor("wk", [HID, DG], F32, kind="ExternalInput").ap()
    wv_d = nc.dram_tensor("wv", [HID, DG], F32, kind="ExternalInput").ap()
    ti_d = nc.dram_tensor("ti", [S, S], F32, kind="ExternalInput").ap()
    mk_d = nc.dram_tensor("mask", [S, S], I32, kind="ExternalInput").ap()
    out_d = nc.dram_tensor("out", [S, DG], F32, kind="ExternalOutput").ap()

    qk_scale = 1.0 / float(np.sqrt(D))
    neg_alpha = -abs(float(alpha))

    with tile.TileContext(nc) as tc, ExitStack() as ctx:
        glob = ctx.enter_context(tc.tile_pool(name="glob", bufs=1))
        idf = glob.tile([128, 128], F32)
        make_identity(nc, idf[:])
        idb = glob.tile([128, 128], BF16)
        make_identity(nc, idb[:])

        big = ctx.enter_context(tc.tile_pool(name="big", bufs=1))
        qt = big.tile([128, NDG * S], F32R)
        kt = big.tile([128, NDG * S], F32R)

        dramp = ctx.enter_context(tc.tile_pool(name="dram", bufs=1, space="DRAM"))
        vd = dramp.tile([NKC, 128, HL65], BF16)

        ps_scr = ctx.enter_context(tc.tile_pool(name="ps_scr", bufs=2, space="PSUM"))
        ps_sT = ctx.enter_context(tc.tile_pool(name="ps_sT", bufs=2, space="PSUM"))
        ps_O = ctx.enter_context(tc.tile_pool(name="ps_O", bufs=1, space="PSUM"))

        # G pools: two single-slot pools alternated across q-groups, plus a
        # small staging pool. Allocated before phase A so the qg=0 G build
        # overlaps the projection phase instead of waiting on SBUF reuse.
        gp1 = ctx.enter_context(tc.tile_pool(name="gp1", bufs=1))
        gst = ctx.enter_context(tc.tile_pool(name="gst", bufs=2))

        NQB = QG // 128
        N_GCHUNK = NQB * NKH

        def build_G(qg, gt):
            gt_v = gt[:].rearrange("p (kc q) -> p kc q", q=QG)
            for i in range(N_GCHUNK):
                build_G_chunk(qg, gt_v, i)

        def build_G_chunk(qg, gt_v, idx):
            # kh-outer order: low-k rows of G complete first so the
            # attention kc loop can start before the whole G is built.
            kh, qb = divmod(idx, NQB)
            q0 = qg * QG + qb * 128
            if True:
                tis = gst.tile([128, KHW], F32, tag="tis", bufs=5)
                nc.sync.dma_start(
                    tis[:], ti_d[q0:q0 + 128, kh * KHW:(kh + 1) * KHW])
                mkb = gst.tile([128, KHW], BF16, tag="mks", bufs=5)
                nc.gpsimd.dma_start(  # casting DMA: int32 -> bf16
                    mkb[:], mk_d[q0:q0 + 128, kh * KHW:(kh + 1) * KHW])
                et = gst.tile([128, KHW], BF16, tag="et")
                nc.scalar.activation(et[:], tis[:], EXP, scale=neg_alpha)
                gq = gst.tile([128, KHW], BF16, tag="gq")
                nc.vector.tensor_mul(gq[:], et[:], mkb[:])
                R4 = min(4, KHW // 128)
                for r in range((KHW // 128) // R4):
                    pt = ps_scr.tile([128, 512], BF16, tag="scr")
                    for j in range(R4):
                        nc.tensor.transpose(
                            pt[:, j * 128:(j + 1) * 128],
                            gq[:, (r * R4 + j) * 128:(r * R4 + j + 1) * 128],
                            idb[:])
                    kc0 = kh * (KHW // 128) + r * R4
                    nc.vector.tensor_copy(
                        gt_v[:, kc0:kc0 + R4, qb * 128:(qb + 1) * 128],
                        pt[:, 0:R4 * 128].rearrange("p (j q) -> p j q", q=128))

        gt0 = gp1.tile([128, NKC * QG], BF16, tag="G")
        gt0_v = gt0[:].rearrange("p (kc q) -> p kc q", q=QG)
        gq_built = 0  # number of qb chunks of G(0) already emitted

        # ---------------- Phase A: projections ----------------
        with tc.tile_pool(name="pa", bufs=1) as pa:
            # stream + round weights (all three resident)
            wrs = {}
            for kind, w_d in (("q", wq_d), ("k", wk_d), ("v", wv_d)):
                wraw = pa.tile([128, NHC * DG], F32, tag="wraw", bufs=1)
                nc.sync.dma_start(
                    wraw[:].rearrange("p (hc n) -> p hc n", n=DG),
                    w_d.rearrange("(hc p) n -> p hc n", p=128))
                wr = pa.tile([128, NHC * DG], F32R, tag="wr_" + kind, bufs=1)
                nc.vector.tensor_copy(wr[:], wraw[:])
                wrs[kind] = wr

            for sg in range(NSG):
                xbt = pa.tile([128, NHC * SGW], F32R, tag="xbt", bufs=1)
                xbt_v = xbt[:].rearrange("p (hc s) -> p hc s", s=SGW)
                for sbl in range(SB_G):
                    xs = pa.tile([128, HID], F32, tag="xs", bufs=2)
                    s0 = sg * SGW + sbl * 128
                    nc.sync.dma_start(xs[:], x_d[s0:s0 + 128, :])
                    for grp in range(NHC // G4):
                        pt = ps_scr.tile([128, 512], F32, tag="scr")
                        for j in range(G4):
                            hc = grp * G4 + j
                            nc.tensor.transpose(
                                pt[:, j * 128:(j + 1) * 128],
                                xs[:, hc * 128:(hc + 1) * 128], idf[:])
                        dst = xbt_v[:, grp * G4:grp * G4 + G4,
                                    sbl * 128:(sbl + 1) * 128]
                        src = pt[:, 0:G4 * 128].rearrange(
                            "p (j q) -> p j q", q=128)
                        nc.vector.tensor_copy(dst, src)
                # projections for this s-group
                for kind in ("q", "k"):
                    n_target = min((2 * sg + (1 if kind == "k" else 0) + 1)
                                   * N_GCHUNK // (2 * NSG), N_GCHUNK)
                    while gq_built < n_target:
                        build_G_chunk(0, gt0_v, gq_built)
                        gq_built += 1
                    wr = wrs[kind]
                    dstT = qt if kind == "q" else kt
                    for dg in range(NDG):
                        pp = ps_scr.tile([128, 512], F32, tag="scr")
                        for hc in range(NHC):
                            nc.tensor.matmul(
                                pp[:, 0:SGW],
                                lhsT=wr[:, hc * DG + dg * 128:
                                        hc * DG + (dg + 1) * 128],
                                rhs=xbt[:, hc * SGW:(hc + 1) * SGW],
                                start=(hc == 0), stop=(hc == NHC - 1))
                        nc.scalar.activation(
                            dstT[:, dg * S + sg * SGW:dg * S + (sg + 1) * SGW],
                            pp[:, 0:SGW], COPY)
                for sbl in range(SB_G):
                    sb = sg * SB_G + sbl
                    pp = ps_scr.tile([128, 512], F32, tag="scr")
                    for hc in range(NHC):
                        nc.tensor.matmul(
                            pp[:, 0:DG],
                            lhsT=xbt[:, hc * SGW + sbl * 128:
                                     hc * SGW + (sbl + 1) * 128],
                            rhs=wrs["v"][:, hc * DG:(hc + 1) * DG],
                            start=(hc == 0), stop=(hc == NHC - 1))
                    vstage = pa.tile([128, HL65], BF16, tag="vstage", bufs=2)
                    vs_v = vstage[:].rearrange("p (h e) -> p h e", e=D + 1)
                    nc.scalar.activation(
                        vs_v[:, :, 0:D],
                        pp[:, 0:DG].rearrange("p (h d) -> p h d", d=D), COPY)
                    nc.gpsimd.memset(vs_v[:, :, D:D + 1], 1.0)
                    nc.sync.dma_start(vd[:][sb], vstage[:])
                # interleave a slice of the G(0) build after each s-group
                n_target = ((sg + 1) * N_GCHUNK) // NSG
                while gq_built < n_target:
                    build_G_chunk(0, gt0_v, gq_built)
                    gq_built += 1

        while gq_built < N_GCHUNK:
            build_G_chunk(0, gt0_v, gq_built)
            gq_built += 1

        # ---------------- Phase B: attention ----------------
        with tc.tile_pool(name="gp2", bufs=1) as gp2, \
             tc.tile_pool(name="vp", bufs=1) as vp, \
             tc.tile_pool(name="pb2", bufs=2) as pb2, \
             tc.tile_pool(name="pb3", bufs=3) as pb3:

            vsb = vp.tile([128, NKC * HL65], BF16)
            for kc in range(NKC):
                nc.sync.dma_start(vsb[:, kc * HL65:(kc + 1) * HL65], vd[:][kc])

            gt_cur = gt0
            for qg in range(NQG):
                for h in range(HL):
                    if h == 1 and qg + 1 < NQG:
                        pool = gp2 if (qg + 1) % 2 else gp1
                        gt_next = pool.tile([128, NKC * QG], BF16, tag="G")
                        build_G(qg + 1, gt_next)
                    dgb = h // HPD
                    poff = (h % HPD) * D
                    o_ps = ps_O.tile([D + 1, QG], F32, tag="O")
                    for kc in range(NKC):
                        s_ps = ps_sT.tile([128, QG], F32, tag="sT")
                        for j in range(NJ):
                            nc.tensor.matmul(
                                s_ps[:, j * JW:(j + 1) * JW],
                                lhsT=kt[poff:poff + D,
                                        dgb * S + kc * 128:dgb * S + (kc + 1) * 128],
                                rhs=qt[poff:poff + D,
                                       dgb * S + qg * QG + j * JW:
                                       dgb * S + qg * QG + (j + 1) * JW],
                                start=True, stop=True)
                        pt = pb3.tile([128, QG], BF16, tag="p")
                        nc.scalar.activation(pt[:], s_ps[:], EXP, scale=qk_scale)
                        pm = pb3.tile([128, QG], BF16, tag="pm")
                        nc.vector.tensor_mul(
                            pm[:], pt[:], gt_cur[:, kc * QG:(kc + 1) * QG])
                        for j in range(NJ):
                            nc.tensor.matmul(
                                o_ps[:, j * JW:(j + 1) * JW],
                                lhsT=vsb[:, kc * HL65 + h * (D + 1):
                                         kc * HL65 + (h + 1) * (D + 1)],
                                rhs=pm[:, j * JW:(j + 1) * JW],
                                start=(kc == 0), stop=(kc == NKC - 1))
                    # drain O': transpose back, normalize, store
                    osb = pb2.tile([D + 1, QG], F32, tag="osb")
                    nc.vector.tensor_copy(osb[:], o_ps[:])
                    ostage = pb2.tile([128, (QG // 128) * D], F32, tag="ostage")
                    Q4 = min(4, QG // 128)
                    for qq in range((QG // 128) // Q4):
                        pt = ps_scr.tile([128, 512], F32, tag="scr")
                        for j in range(Q4):
                            qb = qq * Q4 + j
                            nc.tensor.transpose(
                                pt[:, j * 65:j * 65 + 65],
                                osb[0:D + 1, qb * 128:(qb + 1) * 128],
                                idf[0:D + 1, 0:D + 1])
                        rec = pb2.tile([128, Q4], F32, tag="rec")
                        ptv = pt[:, 0:Q4 * 65].rearrange("p (j c) -> p j c", c=65)
                        nc.vector.reciprocal(rec[:], ptv[:, :, 64])
                        for j in range(Q4):
                            qb = qq * Q4 + j
                            nc.vector.tensor_scalar(
                                out=ostage[:, qb * D:(qb + 1) * D],
                                in0=pt[:, j * 65:j * 65 + D],
                                scalar1=rec[:, j:j + 1], scalar2=None,
                                op0=mybir.AluOpType.mult)
                    out_view = out_d[qg * QG:(qg + 1) * QG,
                                     h * D:(h + 1) * D].rearrange(
                                         "(j p) c -> p j c", p=128)
                    nc.sync.dma_start(
                        out_view,
                        ostage[:].rearrange("p (j c) -> p j c", c=D))
                if qg + 1 < NQG:
                    gt_cur = gt_next

    nc.compile()
    return nc


# ---------------- host side ----------------

B_FULL, S_FULL, HID_FULL = 4, 2048, 1024
HEADS_FULL = 16
D_FULL = HID_FULL // HEADS_FULL
DG_FULL = HID_FULL // 2  # columns per core (8 heads)

_CACHE = {}


def _get_nc(alpha):
    key = round(float(alpha), 10)
    if key not in _CACHE:
        _CACHE[key] = build_nc(S_FULL, HID_FULL, DG_FULL, D_FULL, alpha)
    return _CACHE[key]


def make_in_maps(x, time_intervals, mask, Wq, bq, Wk, bk, Wv, bv, alpha):
    x = np.asarray(x, dtype=np.float32)
    ti = np.asarray(time_intervals, dtype=np.float32)
    mk = np.asarray(mask)
    Wq = np.asarray(Wq, dtype=np.float32)
    Wk = np.asarray(Wk, dtype=np.float32)
    Wv = np.asarray(Wv, dtype=np.float32)
    for b in (bq, bk, bv):
        assert not np.any(np.asarray(b)), "nonzero biases not supported"
    in_maps = []
    for c in range(N_CORES):
        b, g = divmod(c, 2)
        cols = slice(g * DG_FULL, (g + 1) * DG_FULL)
        in_maps.append({
            "x": np.ascontiguousarray(x[b]),
            "wq": np.ascontiguousarray(Wq[:, cols]),
            "wk": np.ascontiguousarray(Wk[:, cols]),
            "wv": np.ascontiguousarray(Wv[:, cols]),
            "ti": np.ascontiguousarray(ti[b]),
            "mask": np.ascontiguousarray(mk[b, 0].astype(np.int32)),
        })
    return in_maps


def gather_out(results):
    out = np.empty((B_FULL, S_FULL, HID_FULL), dtype=np.float32)
    for c in range(N_CORES):
        b, g = divmod(c, 2)
        out[b, :, g * DG_FULL:(g + 1) * DG_FULL] = results[c]["out"]
    return out


def kernel(x, time_intervals, mask, Wq, bq, Wk, bk, Wv, bv, alpha):
    from concourse.bass_utils import run_bass_kernel_spmd
    nc = _get_nc(alpha)
    in_maps = make_in_maps(x, time_intervals, mask, Wq, bq, Wk, bk, Wv, bv, alpha)
    res = run_bass_kernel_spmd(nc, in_maps, core_ids=list(range(N_CORES)))
    return gather_out(res.results)

